# revision 2
# baseline (speedup 1.0000x reference)
"""Trainium2 Bass kernel for nn_BAGDnet: batched gather + pose-projection.

For each measurement n with ids (kf_n, mp_n): out[n] = (q'x/qz, q'y/qz)
where q' = M[kf_n] @ [p_{mp_n}, 1] and M is the 3x4 pose matrix with the
camera intrinsics pre-folded on the host (M0 = FX*T0 + CX*T2,
M1 = FY*T1 + CY*T2, M2 = T2), so no per-element scale/offset remains.

All gathers run on-chip via GPSIMD ap_gather (d=1) from SBUF-resident
tables - no per-measurement DMA descriptors at all (descriptor generation
on the Q7 cores was the previous bottleneck at ~4ns of Pool time per
index; ap_gather serves 16 partitions per index with all 8 Q7 cores in
parallel).

Layout (per device, 8-way data-parallel over the measurement axis):
  - Host routes each measurement to the GPSIMD core c = mp // 12500 that
    owns its point-table shard (K_PAD slots per core), sorted by mp and
    split into BKT equal mp-ranges per core: each point-gather call then
    only spans SHARD/BKT table rows, which measures ~1.5x faster than the
    full-shard span.
  - One SBUF table [128, 12500]: partition 16c+j holds, for j<12, pose
    component j of all 2000 keyframes (zero-padded), and for j in 12..15
    the x/y/z/1 components of point shard c.
  - ap_gather with the kf idx stream reads 12 pose components per index;
    a second call with the bucket-local mp idx stream reads x/y/z/1.
    Output is banded SoA: component j of measurement (c, i) at partition
    16c+j, free position i.
  - PE transposes [128,128] chunks of both gather tiles into PSUM
    (identity matmul), giving per-partition AoS structs: measurement
    (c, 128u+p) at partition p, cols 16c..16c+15.
  - DVE per group of U=8 chunks: pts = B_t[...,12:16] (PSUM->SBUF copy),
    prod = A_t[...,(3,4)] * pts (broadcast), q = reduce_add(prod, X),
    rz = 1/q[...,2], out = q[...,0:2] * rz.
  - Outputs stream back as [128, chunk, band, 2]; host un-permutes.
"""

import sys

sys.path.insert(0, "/opt/trn_rl_repo")

from contextlib import ExitStack

import numpy as np

from concourse import bacc, bass, mybir
import concourse.tile as tile
from concourse.bass_utils import run_bass_kernel_spmd

f32 = mybir.dt.float32
i16 = mybir.dt.int16

FX, FY, CX, CY = 320.0, 320.0, 320.0, 240.0
N_MEAS, N_MP, N_KF = 2_000_000, 100_000, 2_000
N_CORES = 8
PER_CORE = N_MEAS // N_CORES  # 250_000 measurements per device
P = 128
SHARD = N_MP // 8  # 12500 points per GPSIMD core
K_PAD = 32768  # slots per GPSIMD core (mean 31250; bucket-level padding)
KT = 2048  # idx per core per ap_gather call
U = 8  # transposed chunks per PSUM group (2 banks A + 2 banks B, x2 bufs)
BKT = 4  # mp-range buckets per core (gather span SHARD/BKT rows)

LAST_RESULTS = None


def build_program(k_pad=K_PAD, repeat=1):
    n_tiles = k_pad // KT
    n_chunk = k_pad // P
    nc = bacc.Bacc(
        "TRN2",
        target_bir_lowering=False,
        debug=False,
        enable_asserts=False,
    )
    tbl_d = nc.dram_tensor("tbl", [P, SHARD, 1], f32, kind="ExternalInput").ap()
    kf_d = nc.dram_tensor("kf", [P, k_pad // 16], i16, kind="ExternalInput").ap()
    mp_d = nc.dram_tensor("mp", [P, k_pad // 16], i16, kind="ExternalInput").ap()
    ident_d = nc.dram_tensor("ident", [P, P], f32, kind="ExternalInput").ap()
    out_d = nc.dram_tensor("out", [P, n_chunk, 8, 2], f32, kind="ExternalOutput").ap()

    A = mybir.AluOpType

    with tile.TileContext(nc) as tc, ExitStack() as ctx:
        const = ctx.enter_context(tc.tile_pool(name="const", bufs=1))
        ga = ctx.enter_context(tc.tile_pool(name="ga", bufs=2))
        gb = ctx.enter_context(tc.tile_pool(name="gb", bufs=2))
        psa = ctx.enter_context(tc.tile_pool(name="psa", bufs=2, space="PSUM"))
        psb = ctx.enter_context(tc.tile_pool(name="psb", bufs=2, space="PSUM"))
        wp = ctx.enter_context(tc.tile_pool(name="work", bufs=2))
        op = ctx.enter_context(tc.tile_pool(name="out", bufs=2))

        tbl = const.tile([P, SHARD, 1], f32, tag="tbl")
        nc.sync.dma_start(out=tbl[:, :, :], in_=tbl_d)
        kf = const.tile([P, k_pad // 16], i16, tag="kf")
        nc.sync.dma_start(out=kf[:, :], in_=kf_d)
        mp = const.tile([P, k_pad // 16], i16, tag="mp")
        nc.sync.dma_start(out=mp[:, :], in_=mp_d)
        ident = const.tile([P, P], f32, tag="ident")
        nc.sync.dma_start(out=ident[:, :], in_=ident_d)

        S = KT // 16
        bw = SHARD // BKT
        for r in range(repeat):
            for t in range(n_tiles):
                gA = ga.tile([P, KT, 1], f32, tag="gA")
                nc.gpsimd.ap_gather(
                    gA[:, :, :],
                    tbl[:, 0:N_KF, :],
                    kf[:, t * S : (t + 1) * S],
                    channels=P,
                    num_elems=N_KF,
                    d=1,
                    num_idxs=KT,
                )
                gB = gb.tile([P, KT, 1], f32, tag="gB")
                bkt = t // (n_tiles // BKT)
                nc.gpsimd.ap_gather(
                    gB[:, :, :],
                    tbl[:, bkt * bw : (bkt + 1) * bw, :],
                    mp[:, t * S : (t + 1) * S],
                    channels=P,
                    num_elems=bw,
                    d=1,
                    num_idxs=KT,
                )

                ot = op.tile([P, KT // P, 8, 2], f32, tag="ot")
                for g in range(KT // P // U):
                    pA = psa.tile([P, U * P], f32, tag="pA")
                    pB = psb.tile([P, U * P], f32, tag="pB")
                    for u in range(U):
                        cu = g * U + u
                        nc.tensor.matmul(
                            pA[:, u * P : (u + 1) * P],
                            gA[:, cu * P : (cu + 1) * P, 0],
                            ident[:, :],
                            is_transpose=True,
                        )
                        nc.tensor.matmul(
                            pB[:, u * P : (u + 1) * P],
                            gB[:, cu * P : (cu + 1) * P, 0],
                            ident[:, :],
                            is_transpose=True,
                        )
                    vA = pA[:, :].rearrange("p (u b j) -> p u b j", u=U, b=8)
                    vB = pB[:, :].rearrange("p (u b j) -> p u b j", u=U, b=8)
                    # DVE may read only one PSUM operand per instruction:
                    # stage the 4 point components through SBUF.
                    pts = wp.tile([P, U, 8, 4], f32, tag="pts")
                    nc.vector.tensor_copy(
                        out=pts[:, :, :, :], in_=vB[:, :, :, 12:16]
                    )
                    prod = wp.tile([P, U, 8, 3, 4], f32, tag="prod")
                    nc.vector.tensor_tensor(
                        out=prod[:, :, :, :, :],
                        in0=vA[:, :, :, 0:12].rearrange(
                            "p u b (r j) -> p u b r j", r=3
                        ),
                        in1=pts[:, :, :, :]
                        .unsqueeze(3)
                        .to_broadcast([P, U, 8, 3, 4]),
                        op=A.mult,
                    )
                    q3 = wp.tile([P, U, 8, 3], f32, tag="q3")
                    nc.vector.tensor_reduce(
                        out=q3[:, :, :, :],
                        in_=prod[:, :, :, :, :],
                        axis=mybir.AxisListType.X,
                        op=A.add,
                    )
                    rz = wp.tile([P, U, 8], f32, tag="rz")
                    nc.vector.reciprocal(out=rz[:, :, :], in_=q3[:, :, :, 2])
                    nc.vector.tensor_tensor(
                        out=ot[:, g * U : (g + 1) * U, :, :],
                        in0=q3[:, :, :, 0:2],
                        in1=rz[:, :, :].to_broadcast([P, U, 8, 2]),
                        op=A.mult,
                    )
                nc.sync.dma_start(
                    out=out_d[:, t * (KT // P) : (t + 1) * (KT // P), :, :],
                    in_=ot[:, :, :, :],
                )

    nc.compile()
    return nc


_PROGRAM_CACHE = {}


def _get_program(key, builder):
    if key not in _PROGRAM_CACHE:
        _PROGRAM_CACHE[key] = builder()
    return _PROGRAM_CACHE[key]


def _pack_table(tMP, tKF):
    """[128, SHARD] table: partition 16c+j -> pose comp j (j<12, kf-indexed,
    zero-padded to SHARD) / point shard c comps x,y,z,1 (j=12..15)."""
    T = tKF.reshape(N_KF, 4, 4)
    M = np.empty((N_KF, 12), dtype=np.float32)
    M[:, 0:4] = FX * T[:, 0, :] + CX * T[:, 2, :]
    M[:, 4:8] = FY * T[:, 1, :] + CY * T[:, 2, :]
    M[:, 8:12] = T[:, 2, :]
    tbl = np.zeros((P, SHARD), dtype=np.float32)
    for c in range(8):
        tbl[16 * c : 16 * c + 12, :N_KF] = M.T
        sh = tMP[c * SHARD : (c + 1) * SHARD]
        tbl[16 * c + 12, :] = sh[:, 0]
        tbl[16 * c + 13, :] = sh[:, 1]
        tbl[16 * c + 14, :] = sh[:, 2]
        tbl[16 * c + 15, :] = 1.0
    return tbl.reshape(P, SHARD, 1)


def _wrap16(arr):
    """[8, K] per-core idx streams -> [128, K//16] wrapped layout
    (core c's index n lives at partition 16c + n%16, column n//16)."""
    k = arr.shape[1]
    return arr.reshape(8, k // 16, 16).transpose(0, 2, 1).reshape(P, k // 16)


def prepare(measurements, tMP, tKF, idxMP, idxKF):
    measurements = np.asarray(measurements, dtype=np.float32)
    tMP = np.ascontiguousarray(np.asarray(tMP, dtype=np.float32))
    tKF = np.ascontiguousarray(np.asarray(tKF, dtype=np.float32))
    idxMP = np.asarray(idxMP)
    idxKF = np.asarray(idxKF)

    n = measurements.shape[0]
    assert n == N_MEAS, f"kernel compiled for {N_MEAS} measurements, got {n}"

    kf = measurements[:, 0].astype(np.int32)
    mp = measurements[:, 1].astype(np.int32)
    # ids are sorted unique (arange in practice) so the searchsorted join is
    # the identity; otherwise remap on host as a fallback.
    if not (
        idxKF.shape[0] == tKF.shape[0]
        and idxMP.shape[0] == tMP.shape[0]
        and np.array_equal(idxKF, np.arange(idxKF.shape[0], dtype=idxKF.dtype))
        and np.array_equal(idxMP, np.arange(idxMP.shape[0], dtype=idxMP.dtype))
    ):
        kf = np.searchsorted(idxKF, kf).astype(np.int32)
        mp = np.searchsorted(idxMP, mp).astype(np.int32)

    tbl = _pack_table(tMP, tKF)
    ident = np.eye(P, dtype=np.float32)

    bw = SHARD // BKT
    in_maps = []
    orders = []
    counts_all = []
    max_bc = 0
    for d in range(N_CORES):
        sl = slice(d * PER_CORE, (d + 1) * PER_CORE)
        # mp-sort groups measurements by owning core and by bucket within
        # the core (core boundaries are multiples of bw)
        order = np.argsort(mp[sl], kind="stable")
        cb = np.bincount(mp[sl] // bw, minlength=8 * BKT).reshape(8, BKT)
        max_bc = max(max_bc, cb.max())
        orders.append(order)
        counts_all.append(cb)

    # bucket slots must be a multiple of KT and hold the largest bucket
    k_pad = max(K_PAD, BKT * KT * ((max_bc + KT - 1) // KT))
    nc = _get_program(("main", k_pad), lambda: build_program(k_pad=k_pad))
    bslots = k_pad // BKT

    for d in range(N_CORES):
        sl = slice(d * PER_CORE, (d + 1) * PER_CORE)
        kfd, mpd = kf[sl], mp[sl]
        order, cb = orders[d], counts_all[d]
        kf_pad = np.zeros((8, k_pad), dtype=np.int16)
        mp_pad = np.zeros((8, k_pad), dtype=np.int16)
        off = 0
        for c in range(8):
            for b in range(BKT):
                n_cb = cb[c, b]
                seg = order[off : off + n_cb]
                base = b * bslots
                kf_pad[c, base : base + n_cb] = kfd[seg]
                mp_pad[c, base : base + n_cb] = mpd[seg] - c * SHARD - b * bw
                off += n_cb
        in_maps.append(
            {
                "tbl": tbl,
                "kf": _wrap16(kf_pad),
                "mp": _wrap16(mp_pad),
                "ident": ident,
            }
        )
    return nc, in_maps, (orders, counts_all, k_pad)


def _assemble(outs_per_core, meta):
    orders, counts_all, k_pad = meta
    bslots = k_pad // BKT
    full = np.empty((N_MEAS, 2), dtype=np.float32)
    for d, o in enumerate(outs_per_core):
        # o: [128, n_chunk, 8, 2]; meas (core c, slot n) -> o[n%128, n//128, c]
        res = o.transpose(2, 1, 0, 3).reshape(8, k_pad, 2)
        order, cb = orders[d], counts_all[d]
        dst = full[d * PER_CORE : (d + 1) * PER_CORE]
        off = 0
        for c in range(8):
            for b in range(BKT):
                n_cb = cb[c, b]
                seg = order[off : off + n_cb]
                base = b * bslots
                dst[seg] = res[c, base : base + n_cb]
                off += n_cb
    return full


def kernel(measurements, tMP, tKF, idxMP, idxKF, trace=False):
    global LAST_RESULTS
    nc, in_maps, meta = prepare(measurements, tMP, tKF, idxMP, idxKF)
    res = run_bass_kernel_spmd(nc, in_maps, list(range(N_CORES)), trace=trace)
    LAST_RESULTS = res
    return _assemble([res.results[c]["out"] for c in range(N_CORES)], meta)


# ---------------------------------------------------------------------------
# Timing helpers (devloop only; not used by the grading path)
# ---------------------------------------------------------------------------


def _make_runner(nc, n_cores):
    """Jitted no-donation runner so device-resident inputs can be reused
    across calls.  Modeled on bass2jax.run_bass_via_pjrt."""
    import jax
    from jax.sharding import Mesh, PartitionSpec
    from jax.experimental.shard_map import shard_map
    from concourse.bass2jax import (
        _bass_exec_p,
        install_neuronx_cc_hook,
        partition_id_tensor,
    )

    install_neuronx_cc_hook()
    assert nc.dbg_addr is None
    partition_name = (
        nc.partition_id_tensor.name if nc.partition_id_tensor else None
    )

    in_names, out_names, out_avals = [], [], []
    for alloc in nc.m.functions[0].allocations:
        if not isinstance(alloc, mybir.MemoryLocationSet):
            continue
        name = alloc.memorylocations[0].name
        if alloc.kind == "ExternalInput":
            if name != partition_name:
                in_names.append(name)
        elif alloc.kind == "ExternalOutput":
            out_names.append(name)
            out_avals.append(
                jax.core.ShapedArray(
                    tuple(alloc.tensor_shape), mybir.dt.np(alloc.dtype)
                )
            )
    n_params = len(in_names)
    n_outs = len(out_avals)
    all_names = tuple(
        in_names + out_names + ([partition_name] if partition_name else [])
    )

    def _body(*args):
        extra = [partition_id_tensor()] if partition_name else []
        outs = _bass_exec_p.bind(
            *args,
            *extra,
            out_avals=tuple(out_avals),
            in_names=all_names,
            out_names=tuple(out_names),
            lowering_input_output_aliases=(),
            sim_require_finite=True,
            sim_require_nnan=True,
            nc=nc,
        )
        return tuple(outs)

    devices = jax.devices()[:n_cores]
    mesh = Mesh(np.asarray(devices), ("core",))
    specs = (PartitionSpec("core"),) * (n_params + n_outs)
    fn = jax.jit(
        shard_map(
            _body,
            mesh=mesh,
            in_specs=specs,
            out_specs=(PartitionSpec("core"),) * n_outs,
            check_rep=False,
        ),
        keep_unused=True,
    )
    return fn, mesh, in_names, out_names, out_avals


def run_once_timed(nc, in_maps, reps=5):
    import time
    import jax
    from jax.sharding import NamedSharding, PartitionSpec

    fn, mesh, in_names, out_names, out_avals = _make_runner(nc, len(in_maps))
    n_cores = len(in_maps)
    sh = NamedSharding(mesh, PartitionSpec("core"))
    dev_in = [
        jax.device_put(
            np.concatenate([np.asarray(m[name]) for m in in_maps], axis=0), sh
        )
        for name in in_names
    ]
    dev_zero = [
        jax.device_put(
            np.zeros((n_cores * a.shape[0], *a.shape[1:]), a.dtype), sh
        )
        for a in out_avals
    ]
    out = fn(*dev_in, *dev_zero)  # compile + warm
    jax.block_until_ready(out)
    best = float("inf")
    for _ in range(reps):
        t0 = time.perf_counter()
        out = fn(*dev_in, *dev_zero)
        jax.block_until_ready(out)
        t1 = time.perf_counter()
        best = min(best, t1 - t0)
    return best, [np.asarray(o) for o in out]
